# revision 1
# baseline (speedup 1.0000x reference)
"""Multi-head self-attention (S=2048, B=2, D=1024, H=16) on 8 TRN2 NeuronCores.

Sharding: core c handles batch b = c//4 and head-quad g = c%4 (4 heads of 64).
Megatron-style: in_proj column-sliced, out_proj row-sliced; host sums the 8
partial outputs and adds out_proj bias.

Per-core dataflow (matmul inputs bf16, accumulation fp32):
  - host supplies x^T (D-major) activations and pre-transposed weight slices
  - qpT/kpT computed head-major (m on partitions, seq on free)
  - vp computed seq-major with an interleaved ones column per head (65-wide
    blocks) so the PV matmul also produces softmax row-sums on partition 64
  - scores^T per (head-pair, 512-query-chunk, key-tile) in a packed psum tile
    (128, 2, 512); exp on ACT reads the pair in one op
  - normalization: K=1 matmul broadcasts the row-sums, DVE divides
  - out-projection on device from attn^T; bias + cross-core reduction on host
"""

import math
from contextlib import ExitStack, nullcontext as _null_ctx

import numpy as np

S = 2048
B = 2
D = 1024
H = 16
DK = 64
HC = 4          # heads per core
M = HC * DK     # 256 head-dim columns per core
N_CORES = 8
KT = S // 128   # 16 key tiles
QQ = 4          # 512-wide query chunks

MM_DT = "bfloat16"   # dtype of matmul inputs ("bfloat16" or "float32r")

_compiled = None


def _build_program():
    import concourse.tile as tile
    from concourse import mybir, bacc

    f32 = mybir.dt.float32
    f32r = mybir.dt.float32r
    mdt = getattr(mybir.dt, MM_DT)
    EXP = mybir.ActivationFunctionType.Exp

    nc = bacc.Bacc("TRN2", target_bir_lowering=False, debug=False)

    xqT = nc.dram_tensor("xqT", [D, S], mdt, kind="ExternalInput").ap()
    xkT = nc.dram_tensor("xkT", [D, S], mdt, kind="ExternalInput").ap()
    xvT = nc.dram_tensor("xvT", [D, S], mdt, kind="ExternalInput").ap()
    wqT = nc.dram_tensor("wqT", [D, M], mdt, kind="ExternalInput").ap()
    wkT = nc.dram_tensor("wkT", [D, M], mdt, kind="ExternalInput").ap()
    wvT = nc.dram_tensor("wvT", [D, M], mdt, kind="ExternalInput").ap()
    bq = nc.dram_tensor("bq", [M], f32, kind="ExternalInput").ap()
    bk = nc.dram_tensor("bk", [M], f32, kind="ExternalInput").ap()
    bv = nc.dram_tensor("bv", [M], mdt, kind="ExternalInput").ap()
    woT = nc.dram_tensor("woT", [M, D], mdt, kind="ExternalInput").ap()
    ones32_dr = nc.dram_tensor("ones32", [1, 64], f32r, kind="ExternalInput").ap()
    out = nc.dram_tensor("out", [S, D], f32, kind="ExternalOutput").ap()

    with tile.TileContext(nc) as tc, ExitStack() as ctx:
        const_pool = ctx.enter_context(tc.tile_pool(name="const", bufs=1))
        x_pool = ctx.enter_context(tc.tile_pool(name="x", bufs=16))
        xv_pool = ctx.enter_context(tc.tile_pool(name="xv", bufs=16))
        e_pool = ctx.enter_context(tc.tile_pool(name="e", bufs=20))
        o_pool = ctx.enter_context(tc.tile_pool(name="o", bufs=2))
        r_pool = ctx.enter_context(tc.tile_pool(name="r", bufs=2))
        ps_a = ctx.enter_context(tc.tile_pool(name="ps_a", bufs=2, space="PSUM"))
        ps_b = ctx.enter_context(tc.tile_pool(name="ps_b", bufs=4, space="PSUM"))

        # ---- persistent SBUF tensors ----
        # weight slices as matmul lhsT, K-chunked: [p, kc, m]
        wq_sb = const_pool.tile([128, 8, M], mdt)
        wk_sb = const_pool.tile([128, 8, M], mdt)
        wv_sb = const_pool.tile([128, 8, M], mdt)
        for w_sb, w_dr in ((wq_sb, wqT), (wk_sb, wkT), (wv_sb, wvT)):
            nc.sync.dma_start(
                out=w_sb[:, :, :], in_=w_dr.rearrange("(kc p) m -> p kc m", p=128)
            )
        # out_proj rhs: [p, kc, j]
        wo_sb = const_pool.tile([128, 2, D], mdt)
        nc.sync.dma_start(
            out=wo_sb[:, :, :], in_=woT.rearrange("(kc p) j -> p kc j", p=128)
        )
        # per-partition biases for qpT/kpT: [p, mt]
        bq_sb = const_pool.tile([128, 2], f32)
        bk_sb = const_pool.tile([128, 2], f32)
        nc.sync.dma_start(out=bq_sb[:, :], in_=bq.rearrange("(mt p) -> p mt", p=128))
        nc.sync.dma_start(out=bk_sb[:, :], in_=bk.rearrange("(mt p) -> p mt", p=128))
        # bv as a K=1 matmul rhs row
        bv_sb = const_pool.tile([1, M], mdt)
        nc.sync.dma_start(out=bv_sb[:, :], in_=bv.rearrange("(a m) -> a m", a=1))
        ones_sb = const_pool.tile([1, 128], mdt)
        nc.vector.memset(ones_sb[:, :], 1.0)
        ones32_sb = const_pool.tile([1, 64], f32r)
        nc.sync.dma_start(out=ones32_sb[:, :], in_=ones32_dr[:, :])

        qpT = const_pool.tile([128, 2, S], mdt)   # [p, mt, s]
        kpT = const_pool.tile([128, 2, S], mdt)
        vp = const_pool.tile([128, KT, HC * 65], mdt)  # aug: 65-wide per head
        attnT = const_pool.tile([128, 2, S], mdt)

        # ones columns of the augmented V (once; head h at column h*65+64)
        nc.vector.memset(
            vp[:, :, :].rearrange("p kt (h c) -> p kt h c", c=65)[:, :, :, 64:65], 1.0
        )

        # ---- projections ----
        # x^T K-chunks stay resident (x_pool holds all 16 per tensor), so
        # each weight m-tile can be projected independently of load order.
        def load_half(x_dr, half, pool=None, eng=None):
            fs = half * 1024
            chunks = []
            for kc in range(8):
                xt = (pool or x_pool).tile([128, 1024], mdt, tag="xchunk")
                (eng or nc.sync).dma_start(
                    out=xt[:, :], in_=x_dr[kc * 128:(kc + 1) * 128, fs:fs + 1024]
                )
                chunks.append((xt, fs))
            return chunks

        def load_chunks(x_dr, pool=None, eng=None):
            return load_half(x_dr, 0, pool, eng) + load_half(x_dr, 1, pool, eng)

        def proj_half(chunks, w_sb, b_sb, p_sb, mt, half):
            fs = half * 1024
            for nch in range(2):
                ns = nch * 512
                ps = ps_b.tile([128, 512], f32, tag="ps_small", name="ps_p")
                for kc in range(8):
                    nc.tensor.matmul(
                        ps[:, :],
                        w_sb[:, kc, mt * 128:(mt + 1) * 128],
                        chunks[half * 8 + kc][0][:, ns:ns + 512],
                        start=(kc == 0),
                        stop=(kc == 7),
                    )
                nc.vector.tensor_scalar_add(
                    out=p_sb[:, mt, fs + ns:fs + ns + 512],
                    in0=ps[:, :],
                    scalar1=b_sb[:, mt:mt + 1],
                )

        def vp_group(chunks, kt):
            half, st = divmod(kt, 8)
            ps = ps_b.tile([128, 256], f32, tag="ps_small", name="ps_v")
            for kc in range(8):
                nc.tensor.matmul(
                    ps[:, 0:M],
                    chunks[half * 8 + kc][0][:, st * 128:(st + 1) * 128],
                    wv_sb[:, kc, :],
                    start=(kc == 0),
                    stop=False,
                )
            # bias via K=1 ones-row matmul
            nc.tensor.matmul(
                ps[:, 0:M],
                ones_sb[0:1, 0:128],
                bv_sb[0:1, :],
                start=False,
                stop=True,
            )
            nc.vector.tensor_copy(
                out=vp[:, kt, :].rearrange("p (h c) -> p h c", c=65)[:, :, 0:64],
                in_=ps[:, 0:M].rearrange("p (h c) -> p h c", c=64),
            )

        # interleave loads so scores for the first keys can start after just
        # the first half of xk + xq has landed, with xv staged in between so
        # the just-in-time V projection keeps pace with the PV consumers
        # mt0 projections run on freshly-streamed chunks; the x tiles are
        # then re-streamed later for the mt1 projections (cheap DMA, far off
        # the critical path) so the pool stays small and the E runway large.
        chunks_k = load_half(xkT, 0)
        proj_half(chunks_k, wk_sb, bk_sb, kpT, 0, 0)
        chunks_q = load_half(xqT, 0)
        proj_half(chunks_q, wq_sb, bq_sb, qpT, 0, 0)
        chunks_k += load_half(xkT, 1)
        proj_half(chunks_k, wk_sb, bk_sb, kpT, 0, 1)
        chunks_q += load_half(xqT, 1)
        proj_half(chunks_q, wq_sb, bq_sb, qpT, 0, 1)
        chunks_v = load_chunks(xvT, pool=xv_pool)
        chunks_k2 = load_chunks(xkT)
        for half in range(2):
            proj_half(chunks_k2, wk_sb, bk_sb, kpT, 1, half)
        chunks_q2 = load_chunks(xqT)
        for half in range(2):
            proj_half(chunks_q2, wq_sb, bq_sb, qpT, 1, half)

        # ---- attention + out-projection ----
        # The per-engine runtime schedule is static and in-order, so a
        # segment's normalization/out-projection is emitted INSIDE the next
        # segment's kt loop — its DVE-latency chain then overlaps the next
        # segment's compute instead of head-of-line blocking the PE queue.
        def flush_head(pair, qq, u, hh):
            qs = qq * 512
            rs = r_pool.tile([1, 512], f32r, tag="rs")
            with nc.allow_low_precision(reason="softmax denom"):
                nc.vector.tensor_copy(out=rs[:, :], in_=u[64:65, :])
            us = r_pool.tile([64, 512], f32, tag="us")
            nc.vector.tensor_copy(out=us[:, :], in_=u[0:64, :])
            rb = ps_b.tile([64, 512], f32, tag="ps_small")
            nc.tensor.matmul(
                rb[0:64, :], ones32_sb[0:1, 0:64], rs[0:1, :], start=True, stop=True
            )
            rbs = r_pool.tile([64, 512], f32, tag="rbs")
            nc.vector.reciprocal_approx_fast(out=rbs[:, :], in_=rb[0:64, :])
            with nc.allow_low_precision(reason="softmax normalize"):
                nc.vector.tensor_tensor(
                    out=attnT[hh * 64:hh * 64 + 64, pair, qs:qs + 512],
                    in0=us[0:64, :],
                    in1=rbs[0:64, :],
                    op=mybir.AluOpType.mult,
                )

        def outproj_stile(sg):
            ot = o_pool.tile([128, D], f32)
            for nch in range(2):
                ns = nch * 512
                po = ps_b.tile([128, 512], f32, tag="ps_small")
                for kc in range(2):
                    nc.tensor.matmul(
                        po[:, :],
                        attnT[:, kc, sg * 128:(sg + 1) * 128],
                        wo_sb[:, kc, ns:ns + 512],
                        start=(kc == 0),
                        stop=(kc == 1),
                    )
                nc.vector.tensor_copy(out=ot[:, ns:ns + 512], in_=po[:, :])
            nc.sync.dma_start(out=out[sg * 128:(sg + 1) * 128, :], in_=ot[:, :])

        pending_flush = None   # (pair, qq, u_tiles) awaiting normalization
        pending_out = []       # out-projection s-tiles ready to interleave
        for pair in range(2):
            for qq in range(QQ):
                qs = qq * 512
                u_tiles = []
                for h in (2 * pair, 2 * pair + 1):
                    u_tiles.append(
                        ps_b.tile([65, 512], f32, tag="ps_small", name=f"u_{qq}_{h}")
                    )
                for kt in range(KT):
                    ks = kt * 128
                    with tc.high_priority() if pair == 0 else _null_ctx():
                        sc = ps_a.tile([128, 2, 512], f32, tag="ps_main")
                        for hh in range(2):
                            po = hh * 64
                            nc.tensor.matmul(
                                sc[:, hh, :],
                                kpT[po:po + 64, pair, ks:ks + 128],
                                qpT[po:po + 64, pair, qs:qs + 512],
                                start=True,
                                stop=True,
                            )
                        et = e_pool.tile([128, 2, 512], mdt)
                        nc.scalar.activation(out=et[:, :, :], in_=sc[:, :, :], func=EXP)
                    if pair == 0 and qq == 0:
                        # V projection emitted just-in-time for its first consumer
                        vp_group(chunks_v, kt)
                    for hh in range(2):
                        h = 2 * pair + hh
                        nc.tensor.matmul(
                            u_tiles[hh][0:65, :],
                            vp[:, kt, h * 65:(h + 1) * 65],
                            et[:, hh, :],
                            start=(kt == 0),
                            stop=(kt == KT - 1),
                        )
                    # interleave the previous segment's epilogue
                    if pending_flush is not None and kt in (2, 4):
                        p_pair, p_qq, p_u = pending_flush
                        flush_head(p_pair, p_qq, p_u[kt // 2 - 1], kt // 2 - 1)
                        if kt == 4:
                            if p_pair == 1:
                                pending_out.extend(range(p_qq * 4, p_qq * 4 + 4))
                            pending_flush = None
                    elif pending_out and kt in (6, 9, 12, 15):
                        outproj_stile(pending_out.pop(0))
                pending_flush = (pair, qq, u_tiles)
        # tail: last segment's normalization + remaining out-projection
        p_pair, p_qq, p_u = pending_flush
        flush_head(p_pair, p_qq, p_u[0], 0)
        flush_head(p_pair, p_qq, p_u[1], 1)
        pending_out.extend(range(p_qq * 4, p_qq * 4 + 4))
        for sg in pending_out:
            outproj_stile(sg)

    nc.compile()
    return nc


def _get_compiled():
    global _compiled
    if _compiled is None:
        _compiled = _build_program()
    return _compiled


def _make_in_maps(q, k, v, in_proj_w, in_proj_b, out_proj_w):
    import ml_dtypes

    mdt_np = np.dtype(ml_dtypes.bfloat16) if MM_DT == "bfloat16" else np.float32

    def cvt(a):
        return np.ascontiguousarray(a).astype(mdt_np)

    xT = {}
    for b in range(B):
        xT[b] = (
            cvt(q[:, b, :].T),
            cvt(k[:, b, :].T),
            cvt(v[:, b, :].T),
        )
    scale = 1.0 / math.sqrt(DK)
    in_maps = []
    for c in range(N_CORES):
        b, g = divmod(c, HC)
        cols = slice(g * M, (g + 1) * M)
        in_maps.append({
            "xqT": xT[b][0],
            "xkT": xT[b][1],
            "xvT": xT[b][2],
            "wqT": cvt((in_proj_w[0 * D:1 * D][cols] * scale).T),
            "wkT": cvt(in_proj_w[1 * D:2 * D][cols].T),
            "wvT": cvt(in_proj_w[2 * D:3 * D][cols].T),
            "bq": np.ascontiguousarray(in_proj_b[0 * D:1 * D][cols] * scale),
            "bk": np.ascontiguousarray(in_proj_b[1 * D:2 * D][cols]),
            "bv": cvt(in_proj_b[2 * D:3 * D][cols]),
            "woT": cvt(out_proj_w[:, g * M:(g + 1) * M].T),
            "ones32": np.ones((1, 64), dtype=np.float32),
        })
    return in_maps


def kernel(q, k, v, in_proj_w, in_proj_b, out_proj_w, out_proj_b):
    from concourse.bass_utils import run_bass_kernel_spmd

    q = np.asarray(q, dtype=np.float32)
    k = np.asarray(k, dtype=np.float32)
    v = np.asarray(v, dtype=np.float32)
    in_proj_w = np.asarray(in_proj_w, dtype=np.float32)
    in_proj_b = np.asarray(in_proj_b, dtype=np.float32)
    out_proj_w = np.asarray(out_proj_w, dtype=np.float32)
    out_proj_b = np.asarray(out_proj_b, dtype=np.float32)

    nc = _get_compiled()
    in_maps = _make_in_maps(q, k, v, in_proj_w, in_proj_b, out_proj_w)

    res = run_bass_kernel_spmd(nc, in_maps, core_ids=list(range(N_CORES)))

    out = np.broadcast_to(out_proj_b.astype(np.float32), (S, B, D)).copy()
    for c in range(N_CORES):
        out[:, c // HC, :] += res.results[c]["out"]
    return out



# revision 43
# speedup vs baseline: 1.0109x; 1.0109x over previous
"""Multi-head self-attention (S=2048, B=2, D=1024, H=16) on 8 TRN2 NeuronCores.

Sharding: core c handles batch b = c//4 and head-quad g = c%4 (4 heads of 64).
Megatron-style: in_proj column-sliced, out_proj row-sliced; host sums the 8
partial outputs and adds out_proj bias.

Per-core dataflow (matmul inputs bf16, accumulation fp32):
  - host supplies x^T (D-major) activations and pre-transposed weight slices
  - qpT/kpT computed head-major (m on partitions, seq on free)
  - vp computed seq-major with an interleaved ones column per head (65-wide
    blocks) so the PV matmul also produces softmax row-sums on partition 64
  - scores per (head-pair, 512-query-chunk, key-tile): the two heads' K=64
    matmuls land on disjoint PE row-groups (base partitions 0/64) and run
    concurrently; exp on ACT reads the pair in one op
  - the attention loop is paced by the ACT exp (~1.1us per key-tile); all
    other PE work (V projection, mt1 projections, out-projection, flushes)
    is drip-fed as "filler" between score matmuls via an explicit
    budget-tracked queue so neither engine sees a long stall
  - normalization: K=1 matmul broadcasts the row-sums, DVE divides reading
    the PV accumulator straight from PSUM
  - out-projection on device from attn^T; bias + cross-core reduction on host
"""

import math
from collections import deque
from contextlib import ExitStack

import numpy as np

S = 2048
B = 2
D = 1024
H = 16
DK = 64
HC = 4          # heads per core
M = HC * DK     # 256 head-dim columns per core
N_CORES = 8
KT = S // 128   # 16 key tiles
QQ = 4          # 512-wide query chunks
NSEG = 2 * QQ   # (pair, qq) segments

MM_DT = "bfloat16"

# pacing ledger: target PE-ns budget per attention iteration
PACE = 1280.0
SLOP = 700.0
COST_SCORES = 360.0
COST_PV = 480.0
COST_VP = 990.0
COST_PROJ = 1740.0
COST_RB = 450.0
COST_OUT = 900.0

_compiled = None


def _build_program():
    import concourse.tile as tile
    from concourse import mybir, bacc

    f32 = mybir.dt.float32
    f32r = mybir.dt.float32r
    mdt = getattr(mybir.dt, MM_DT)
    EXP = mybir.ActivationFunctionType.Exp

    nc = bacc.Bacc("TRN2", target_bir_lowering=False, debug=False)

    xqT = nc.dram_tensor("xqT", [D, S], mdt, kind="ExternalInput").ap()
    xkT = nc.dram_tensor("xkT", [D, S], mdt, kind="ExternalInput").ap()
    xvT = nc.dram_tensor("xvT", [D, S], mdt, kind="ExternalInput").ap()
    wqT = nc.dram_tensor("wqT", [D, M], mdt, kind="ExternalInput").ap()
    wkT = nc.dram_tensor("wkT", [D, M], mdt, kind="ExternalInput").ap()
    wvT = nc.dram_tensor("wvT", [D, M], mdt, kind="ExternalInput").ap()
    bq = nc.dram_tensor("bq", [M], f32, kind="ExternalInput").ap()
    bk = nc.dram_tensor("bk", [M], f32, kind="ExternalInput").ap()
    bv = nc.dram_tensor("bv", [M], mdt, kind="ExternalInput").ap()
    woT = nc.dram_tensor("woT", [M, D], mdt, kind="ExternalInput").ap()
    ones32_dr = nc.dram_tensor("ones32", [33, 64], f32r, kind="ExternalInput").ap()
    out = nc.dram_tensor("out", [S, D], f32, kind="ExternalOutput").ap()

    with tile.TileContext(nc) as tc, ExitStack() as ctx:
        const_pool = ctx.enter_context(tc.tile_pool(name="const", bufs=1))
        x_pool = ctx.enter_context(tc.tile_pool(name="x", bufs=32))
        xv_pool = ctx.enter_context(tc.tile_pool(name="xv", bufs=16))
        e_pool = ctx.enter_context(tc.tile_pool(name="e", bufs=16))
        o_pool = ctx.enter_context(tc.tile_pool(name="o", bufs=2))
        r_pool = ctx.enter_context(tc.tile_pool(name="r", bufs=2))
        ps_a = ctx.enter_context(tc.tile_pool(name="ps_a", bufs=2, space="PSUM"))
        ps_b = ctx.enter_context(tc.tile_pool(name="ps_b", bufs=4, space="PSUM"))

        # ---- persistent SBUF tensors ----
        # wo is loaded LAST (after all x tensors) — it isn't needed until the
        # first out-projection, ~100us in.
        wq_sb = const_pool.tile([128, 8, M], mdt)
        wk_sb = const_pool.tile([128, 8, M], mdt)
        wv_sb = const_pool.tile([128, 8, M], mdt)
        for w_sb, w_dr in ((wk_sb, wkT), (wq_sb, wqT), (wv_sb, wvT)):
            nc.sync.dma_start(
                out=w_sb[:, :, :], in_=w_dr.rearrange("(kc p) m -> p kc m", p=128)
            )
        wo_sb = const_pool.tile([128, 2, D], mdt)
        bq_sb = const_pool.tile([128, 2], f32)
        bk_sb = const_pool.tile([128, 2], f32)
        nc.sync.dma_start(out=bq_sb[:, :], in_=bq.rearrange("(mt p) -> p mt", p=128))
        nc.sync.dma_start(out=bk_sb[:, :], in_=bk.rearrange("(mt p) -> p mt", p=128))
        bv_sb = const_pool.tile([1, M], mdt)
        nc.sync.dma_start(out=bv_sb[:, :], in_=bv.rearrange("(a m) -> a m", a=1))
        ones_sb = const_pool.tile([1, 128], mdt)
        nc.vector.memset(ones_sb[:, :], 1.0)
        ones33_sb = const_pool.tile([33, 64], f32r)
        nc.sync.dma_start(out=ones33_sb[:, :], in_=ones32_dr[:, :])

        qpT = const_pool.tile([128, 2, S], mdt)   # [p, mt, s]
        kpT = const_pool.tile([128, 2, S], mdt)
        vp = const_pool.tile([128, KT, HC * 65], mdt)
        attnT = const_pool.tile([128, 2, S], mdt)

        nc.vector.memset(
            vp[:, :, :].rearrange("p kt (h c) -> p kt h c", c=65)[:, :, :, 64:65], 1.0
        )

        # warm the ACT exp table during the prologue DMAs
        warm_sb = const_pool.tile([128, 8], f32)
        nc.vector.memset(warm_sb[:, :], 0.0)
        nc.scalar.activation(out=warm_sb[:, :], in_=warm_sb[:, :], func=EXP)

        # ---- projection / attention helpers ----
        def load_half(x_dr, half, pool=None, defer=False):
            fs = half * 1024
            chunks = []
            for kc in range(8):
                xt = (pool or x_pool).tile([128, 1024], mdt, tag="xchunk")
                if not defer:
                    nc.sync.dma_start(
                        out=xt[:, :], in_=x_dr[kc * 128:(kc + 1) * 128, fs:fs + 1024]
                    )
                chunks.append(xt)
            return chunks

        def load_cols(x_dr, chunks, half, nch):
            # one 512-token column block of every chunk — lets an nch0-only
            # projection (and hence the first score matmul) start earlier
            fs = half * 1024 + nch * 512
            for kc, xt in enumerate(chunks):
                nc.sync.dma_start(
                    out=xt[:, nch * 512:nch * 512 + 512],
                    in_=x_dr[kc * 128:(kc + 1) * 128, fs:fs + 512],
                )

        def proj_nch(chunks, w_sb, b_sb, p_sb, mt, half, nch):
            # one 512-token column block of qpT/kpT: 8 accumulating matmuls
            fs = half * 1024
            ns = nch * 512
            ps = ps_b.tile([128, 512], f32, tag="ps_small", name="ps_p")
            for kc in range(8):
                nc.tensor.matmul(
                    ps[:, :],
                    w_sb[:, kc, mt * 128:(mt + 1) * 128],
                    chunks[kc][:, ns:ns + 512],
                    start=(kc == 0),
                    stop=(kc == 7),
                )
            nc.vector.tensor_scalar_add(
                out=p_sb[:, mt, fs + ns:fs + ns + 512],
                in0=ps[:, :],
                scalar1=b_sb[:, mt:mt + 1],
            )

        def proj_pair(chunks, w_sb, b_sb, p_sb, mt, half):
            # both 512-column blocks of one (tensor, mt, half) projection as
            # two interleaved PSUM chains: each matmul's drain hides behind
            # the other chain's fill
            fs = half * 1024
            pss = [
                ps_b.tile([128, 512], f32, tag="ps_small", name="ps_p")
                for _ in range(2)
            ]
            for kc in range(8):
                for nch in range(2):
                    nc.tensor.matmul(
                        pss[nch][:, :],
                        w_sb[:, kc, mt * 128:(mt + 1) * 128],
                        chunks[kc][:, nch * 512:nch * 512 + 512],
                        start=(kc == 0),
                        stop=(kc == 7),
                    )
            for nch in range(2):
                nc.vector.tensor_scalar_add(
                    out=p_sb[:, mt, fs + nch * 512:fs + nch * 512 + 512],
                    in0=pss[nch][:, :],
                    scalar1=b_sb[:, mt:mt + 1],
                )

        def vp_group(chunks, kt):
            st = kt % 8
            ps = ps_b.tile([128, 256], f32, tag="ps_small", name="ps_v")
            for kc in range(8):
                nc.tensor.matmul(
                    ps[:, 0:M],
                    chunks[kc][:, st * 128:(st + 1) * 128],
                    wv_sb[:, kc, :],
                    start=(kc == 0),
                    stop=False,
                )
            nc.tensor.matmul(
                ps[:, 0:M],
                ones_sb[0:1, 0:128],
                bv_sb[0:1, :],
                start=False,
                stop=True,
            )
            nc.vector.tensor_copy(
                out=vp[:, kt, :].rearrange("p (h c) -> p h c", c=65)[:, :, 0:64],
                in_=ps[:, 0:M].rearrange("p (h c) -> p h c", c=64),
            )

        def flush_pair(pair, qq, uA, uB):
            # both heads' normalization in one item; the two K=1 row-sum
            # broadcast matmuls sit on disjoint PE row-groups (rows 0 / 32
            # via base partitions) and run concurrently
            qs = qq * 512
            rsp = r_pool.tile([33, 512], f32r, tag="rs")
            us2 = [r_pool.tile([64, 512], f32, tag=f"us{i}", name=f"us{i}")
                   for i in range(2)]
            with nc.allow_low_precision(reason="softmax denom"):
                nc.vector.tensor_copy(out=rsp[0:1, :], in_=uA[64:65, :])
                nc.vector.tensor_copy(out=rsp[32:33, :], in_=uB[64:65, :])
            nc.vector.tensor_copy(out=us2[0][:, :], in_=uA[0:64, :])
            nc.vector.tensor_copy(out=us2[1][:, :], in_=uB[0:64, :])
            rbs2 = []
            for hh in range(2):
                rb = ps_b.tile([64, 512], f32, tag="ps_small", name=f"rb{hh}")
                nc.tensor.matmul(
                    rb[0:64, :],
                    ones33_sb[32 * hh:32 * hh + 1, 0:64],
                    rsp[32 * hh:32 * hh + 1, :],
                    start=True,
                    stop=True,
                )
                rbs = r_pool.tile([64, 512], f32, tag=f"rbs{hh}", name=f"rbs{hh}")
                nc.vector.reciprocal_approx_fast(out=rbs[:, :], in_=rb[0:64, :])
                rbs2.append(rbs)
            with nc.allow_low_precision(reason="softmax normalize"):
                for hh in range(2):
                    nc.vector.tensor_tensor(
                        out=attnT[hh * 64:hh * 64 + 64, pair, qs:qs + 512],
                        in0=us2[hh][0:64, :],
                        in1=rbs2[hh][0:64, :],
                        op=mybir.AluOpType.mult,
                    )

        def flush_head(pair, qq, u, hh):
            # NOTE: u must be fully copied out of PSUM before the rb matmul —
            # rb's ring slot may wrap onto u's, and a later read of u would
            # deadlock against rb's WAR dependency.
            qs = qq * 512
            rs = r_pool.tile([1, 512], f32r, tag="rs")
            with nc.allow_low_precision(reason="softmax denom"):
                nc.vector.tensor_copy(out=rs[:, :], in_=u[64:65, :])
            us = r_pool.tile([64, 512], f32, tag="us")
            nc.vector.tensor_copy(out=us[:, :], in_=u[0:64, :])
            rb = ps_b.tile([64, 512], f32, tag="ps_small", name="rb")
            nc.tensor.matmul(
                rb[0:64, :], ones33_sb[0:1, 0:64], rs[0:1, :], start=True, stop=True
            )
            rbs = r_pool.tile([64, 512], f32, tag="rbs")
            nc.vector.reciprocal_approx_fast(out=rbs[:, :], in_=rb[0:64, :])
            with nc.allow_low_precision(reason="softmax normalize"):
                nc.vector.tensor_tensor(
                    out=attnT[hh * 64:hh * 64 + 64, pair, qs:qs + 512],
                    in0=us[0:64, :],
                    in1=rbs[0:64, :],
                    op=mybir.AluOpType.mult,
                )

        def outproj_stile(sg):
            # the two 512-column accumulation chains interleave so each
            # matmul's drain hides behind the other chain's fill
            ot = o_pool.tile([128, D], f32, name="ot")
            pos = [
                ps_b.tile([128, 512], f32, tag="ps_small", name="po")
                for _ in range(2)
            ]
            for kc in range(2):
                for nch in range(2):
                    nc.tensor.matmul(
                        pos[nch][:, :],
                        attnT[:, kc, sg * 128:(sg + 1) * 128],
                        wo_sb[:, kc, nch * 512:nch * 512 + 512],
                        start=(kc == 0),
                        stop=(kc == 1),
                    )
            for nch in range(2):
                nc.vector.tensor_copy(
                    out=ot[:, nch * 512:nch * 512 + 512], in_=pos[nch][:, :]
                )
            nc.sync.dma_start(out=out[sg * 128:(sg + 1) * 128, :], in_=ot[:, :])

        # ---- prologue: minimal path to the first score matmul ----
        # (keys 0-511 then queries 0-511 land first: scores kt0-3 can start
        # after only ~2MB of DMA)
        chunks_k = load_half(xkT, 0, defer=True)
        chunks_q = load_half(xqT, 0, defer=True)
        load_cols(xkT, chunks_k, 0, 0)
        proj_nch(chunks_k, wk_sb, bk_sb, kpT, 0, 0, 0)
        load_cols(xqT, chunks_q, 0, 0)
        proj_nch(chunks_q, wq_sb, bq_sb, qpT, 0, 0, 0)   # queries 0-511
        load_cols(xkT, chunks_k, 0, 1)
        proj_nch(chunks_k, wk_sb, bk_sb, kpT, 0, 0, 1)
        load_cols(xqT, chunks_q, 0, 1)
        chunks_k2 = load_half(xkT, 1)
        chunks_v = load_half(xvT, 0, pool=xv_pool)
        chunks_v2 = load_half(xvT, 1, pool=xv_pool)
        chunks_q2 = load_half(xqT, 1)
        nc.sync.dma_start(
            out=wo_sb[:, :, :], in_=woT.rearrange("(kc p) j -> p kc j", p=128)
        )

        # ---- filler queues ----
        # mand: order-critical chain (V-proj groups -> PV -> flush -> outproj)
        # bg:   independent projection work with (ready, deadline) windows
        mand = deque()   # items: dict(ready, cost, fn, after)
        bg = deque()     # items: dict(ready, deadline, cost, fn)
        pe_ns = [0.0]
        u_tiles = {}
        ets = {}

        pv_unlock = {0: 0}   # seg -> iteration when its PV chain may start

        def m_item(ready, cost, fn, after=None, gate_seg=None, front=False):
            it = {"ready": ready, "cost": cost, "fn": fn, "after": after,
                  "gate_seg": gate_seg}
            if front:
                mand.appendleft(it)
            else:
                mand.append(it)

        def b_item(ready, deadline, cost, fn):
            bg.append({"ready": ready, "deadline": deadline, "cost": cost, "fn": fn})

        def emit_scores_exp(seg, kt):
            pair, qq = divmod(seg, QQ)
            qs, ks = qq * 512, kt * 128
            sc = ps_a.tile([128, 2, 512], f32, tag="ps_main", name="sc")
            for hh in range(2):
                po = hh * 64
                nc.tensor.matmul(
                    sc[:, hh, :],
                    kpT[po:po + 64, pair, ks:ks + 128],
                    qpT[po:po + 64, pair, qs:qs + 512],
                    start=True,
                    stop=True,
                )
            et = e_pool.tile([128, 2, 512], mdt, tag="et", name="et")
            nc.scalar.activation(out=et[:, :, :], in_=sc[:, :, :], func=EXP)
            ets[(seg, kt)] = et
            pe_ns[0] += COST_SCORES

        def emit_pv(seg, kt):
            pair = seg // QQ
            if kt == 0:
                u_tiles[seg] = [
                    ps_b.tile([65, 512], f32, tag="ps_small", name=f"u_{seg}_{h}")
                    for h in range(2)
                ]
            et = ets.pop((seg, kt))
            for hh in range(2):
                h = 2 * pair + hh
                nc.tensor.matmul(
                    u_tiles[seg][hh][0:65, :],
                    vp[:, kt, h * 65:(h + 1) * 65],
                    et[:, hh, :],
                    start=(kt == 0),
                    stop=(kt == KT - 1),
                )

        def push_flushes(seg, g):
            # the flush goes to the queue FRONT: the next segment's PV chain
            # is gated on it (only 2 u accumulators may be live in the PSUM
            # ring at once, else the rb allocation deadlocks on a slot whose
            # holder transitively depends on rb).
            pair, qq = divmod(seg, QQ)

            def fl(p=pair, q=qq, s=seg):
                flush_pair(p, q, u_tiles[s][0], u_tiles[s][1])

            def unlock(g2, s=seg, p=pair, q=qq):
                pv_unlock[s + 1] = g2 + 1
                if p == 1:
                    push_outproj(q, g2)

            m_item(g + 1, 2 * COST_RB, fl, after=unlock, front=True)

        def push_outproj(qq, g):
            for i in range(4):
                m_item(g + 1 + i, COST_OUT,
                       (lambda sg=qq * 4 + i: outproj_stile(sg)))

        # V-projection groups + the PVs of segment 0, in consumption order.
        for kt in range(KT):
            ready = (5 + kt) if kt < 8 else (12 + (kt - 8))
            m_item(ready, COST_VP,
                   (lambda k=kt: vp_group(chunks_v if k < 8 else chunks_v2, k)))
            after = (lambda g: push_flushes(0, g)) if kt == KT - 1 else None
            m_item(ready, COST_PV, (lambda k=kt: emit_pv(0, k)), after=after,
                   gate_seg=0 if kt == 0 else None)

        # background projection fillers (x chunks stay resident in the
        # 32-slot ring, so mt1 reuses them — no re-streaming)
        b_item(0, 10, COST_PROJ,
               lambda: proj_nch(chunks_q, wq_sb, bq_sb, qpT, 0, 0, 1))
        b_item(3, 8, 2 * COST_PROJ,
               lambda: proj_pair(chunks_k2, wk_sb, bk_sb, kpT, 0, 1))
        b_item(16, 26, 2 * COST_PROJ,
               lambda: proj_pair(chunks_q2, wq_sb, bq_sb, qpT, 0, 1))
        for j, half in enumerate((0, 1)):
            b_item(24 + 4 * j, 52 + 4 * j, 2 * COST_PROJ,
                   (lambda h=half:
                    proj_pair(chunks_k if h == 0 else chunks_k2,
                              wk_sb, bk_sb, kpT, 1, h)))
        for j, half in enumerate((0, 1)):
            b_item(34 + 4 * j, 58 + 4 * j, 2 * COST_PROJ,
                   (lambda h=half:
                    proj_pair(chunks_q if h == 0 else chunks_q2,
                              wq_sb, bq_sb, qpT, 1, h)))

        def m_ready(it, g):
            if it["ready"] > g:
                return False
            gs = it["gate_seg"]
            if gs is not None and (gs not in pv_unlock or pv_unlock[gs] > g):
                return False
            return True

        def pump(g, budget):
            progressed = True
            while progressed:
                progressed = False
                if mand and m_ready(mand[0], g) and (
                    pe_ns[0] + mand[0]["cost"] <= budget
                ):
                    it = mand.popleft()
                    it["fn"]()
                    pe_ns[0] += it["cost"]
                    if it["after"]:
                        it["after"](g)
                    progressed = True
                    continue
                if bg and bg[0]["ready"] <= g and (
                    pe_ns[0] + bg[0]["cost"] <= budget or g >= bg[0]["deadline"]
                ):
                    it = bg.popleft()
                    it["fn"]()
                    pe_ns[0] += it["cost"]
                    progressed = True

        # ---- the ACT-paced attention loop ----
        for g in range(NSEG * KT):
            seg, kt = divmod(g, KT)
            emit_scores_exp(seg, kt)
            if seg > 0:
                after = (lambda gg, s=seg: push_flushes(s, gg)) if kt == KT - 1 else None
                m_item(g + 1, COST_PV, (lambda s=seg, k=kt: emit_pv(s, k)),
                       after=after, gate_seg=seg if kt == 0 else None)
            pump(g, (g + 1) * PACE + SLOP)

        # ---- tail: drain remaining work ----
        g = NSEG * KT
        while mand or bg:
            pump(g, float("inf"))
            g += 1

    nc.compile()
    return nc


def _get_compiled():
    global _compiled
    if _compiled is None:
        _compiled = _build_program()
    return _compiled


def _make_in_maps(q, k, v, in_proj_w, in_proj_b, out_proj_w):
    import ml_dtypes

    mdt_np = np.dtype(ml_dtypes.bfloat16) if MM_DT == "bfloat16" else np.float32

    def cvt(a):
        return np.ascontiguousarray(a).astype(mdt_np)

    xT = {}
    for b in range(B):
        xT[b] = (
            cvt(q[:, b, :].T),
            cvt(k[:, b, :].T),
            cvt(v[:, b, :].T),
        )
    scale = 1.0 / math.sqrt(DK)
    in_maps = []
    for c in range(N_CORES):
        b, g = divmod(c, HC)
        cols = slice(g * M, (g + 1) * M)
        in_maps.append({
            "xqT": xT[b][0],
            "xkT": xT[b][1],
            "xvT": xT[b][2],
            "wqT": cvt((in_proj_w[0 * D:1 * D][cols] * scale).T),
            "wkT": cvt(in_proj_w[1 * D:2 * D][cols].T),
            "wvT": cvt(in_proj_w[2 * D:3 * D][cols].T),
            "bq": np.ascontiguousarray(in_proj_b[0 * D:1 * D][cols] * scale),
            "bk": np.ascontiguousarray(in_proj_b[1 * D:2 * D][cols]),
            "bv": cvt(in_proj_b[2 * D:3 * D][cols]),
            "woT": cvt(out_proj_w[:, g * M:(g + 1) * M].T),
            "ones32": np.ones((33, 64), dtype=np.float32),
        })
    return in_maps


def kernel(q, k, v, in_proj_w, in_proj_b, out_proj_w, out_proj_b):
    from concourse.bass_utils import run_bass_kernel_spmd

    q = np.asarray(q, dtype=np.float32)
    k = np.asarray(k, dtype=np.float32)
    v = np.asarray(v, dtype=np.float32)
    in_proj_w = np.asarray(in_proj_w, dtype=np.float32)
    in_proj_b = np.asarray(in_proj_b, dtype=np.float32)
    out_proj_w = np.asarray(out_proj_w, dtype=np.float32)
    out_proj_b = np.asarray(out_proj_b, dtype=np.float32)

    nc = _get_compiled()
    in_maps = _make_in_maps(q, k, v, in_proj_w, in_proj_b, out_proj_w)

    res = run_bass_kernel_spmd(nc, in_maps, core_ids=list(range(N_CORES)))

    out = np.broadcast_to(out_proj_b.astype(np.float32), (S, B, D)).copy()
    for c in range(N_CORES):
        out[:, c // HC, :] += res.results[c]["out"]
    return out


# revision 44
# speedup vs baseline: 1.0482x; 1.0369x over previous
"""Multi-head self-attention (S=2048, B=2, D=1024, H=16) on 8 TRN2 NeuronCores.

Sharding: core c handles batch b = c//4 and head-quad g = c%4 (4 heads of 64).
Megatron-style: in_proj column-sliced, out_proj row-sliced; host sums the 8
partial outputs and adds out_proj bias.

Per-core dataflow (matmul inputs bf16, accumulation fp32):
  - host supplies x^T (D-major) activations and pre-transposed weight slices
  - qpT/kpT computed head-major (m on partitions, seq on free)
  - vp computed seq-major with an interleaved ones column per head (65-wide
    blocks) so the PV matmul also produces softmax row-sums on partition 64
  - scores per (head-pair, 512-query-chunk, key-tile): the two heads' K=64
    matmuls land on disjoint PE row-groups (base partitions 0/64) and run
    concurrently; exp on ACT reads the pair in one op
  - the attention loop is paced by the ACT exp (~1.1us per key-tile); all
    other PE work (V projection, mt1 projections, out-projection, flushes)
    is drip-fed as "filler" between score matmuls via an explicit
    budget-tracked queue so neither engine sees a long stall
  - normalization: K=1 matmul broadcasts the row-sums, DVE divides reading
    the PV accumulator straight from PSUM
  - out-projection on device from attn^T; bias + cross-core reduction on host
"""

import math
from collections import deque
from contextlib import ExitStack

import numpy as np

S = 2048
B = 2
D = 1024
H = 16
DK = 64
HC = 4          # heads per core
M = HC * DK     # 256 head-dim columns per core
N_CORES = 8
KT = S // 128   # 16 key tiles
QQ = 4          # 512-wide query chunks
NSEG = 2 * QQ   # (pair, qq) segments

MM_DT = "bfloat16"

# pacing ledger: target PE-ns budget per attention iteration
PACE = 1280.0
SLOP = 700.0
COST_SCORES = 360.0
COST_PV = 480.0
COST_VP = 990.0
COST_PROJ = 1740.0
COST_RB = 450.0
COST_OUT = 900.0

_compiled = None


def _build_program():
    import concourse.tile as tile
    from concourse import mybir, bacc

    f32 = mybir.dt.float32
    f32r = mybir.dt.float32r
    mdt = getattr(mybir.dt, MM_DT)
    EXP = mybir.ActivationFunctionType.Exp

    nc = bacc.Bacc("TRN2", target_bir_lowering=False, debug=False)

    xqT = nc.dram_tensor("xqT", [D, S], mdt, kind="ExternalInput").ap()
    xkT = nc.dram_tensor("xkT", [D, S], mdt, kind="ExternalInput").ap()
    xvT = nc.dram_tensor("xvT", [D, S], mdt, kind="ExternalInput").ap()
    wqT = nc.dram_tensor("wqT", [D, M], mdt, kind="ExternalInput").ap()
    wkT = nc.dram_tensor("wkT", [D, M], mdt, kind="ExternalInput").ap()
    wvT = nc.dram_tensor("wvT", [D, M], mdt, kind="ExternalInput").ap()
    bq = nc.dram_tensor("bq", [M], f32, kind="ExternalInput").ap()
    bk = nc.dram_tensor("bk", [M], f32, kind="ExternalInput").ap()
    bv = nc.dram_tensor("bv", [M], mdt, kind="ExternalInput").ap()
    woT = nc.dram_tensor("woT", [M, D], mdt, kind="ExternalInput").ap()
    ones32_dr = nc.dram_tensor("ones32", [1, 64], f32r, kind="ExternalInput").ap()
    out = nc.dram_tensor("out", [S, D], f32, kind="ExternalOutput").ap()

    with tile.TileContext(nc) as tc, ExitStack() as ctx:
        const_pool = ctx.enter_context(tc.tile_pool(name="const", bufs=1))
        x_pool = ctx.enter_context(tc.tile_pool(name="x", bufs=32))
        xv_pool = ctx.enter_context(tc.tile_pool(name="xv", bufs=16))
        e_pool = ctx.enter_context(tc.tile_pool(name="e", bufs=16))
        o_pool = ctx.enter_context(tc.tile_pool(name="o", bufs=2))
        r_pool = ctx.enter_context(tc.tile_pool(name="r", bufs=2))
        ps_a = ctx.enter_context(tc.tile_pool(name="ps_a", bufs=2, space="PSUM"))
        ps_b = ctx.enter_context(tc.tile_pool(name="ps_b", bufs=4, space="PSUM"))

        # ---- persistent SBUF tensors ----
        # wo is loaded LAST (after all x tensors) — it isn't needed until the
        # first out-projection, ~100us in.
        wq_sb = const_pool.tile([128, 8, M], mdt)
        wk_sb = const_pool.tile([128, 8, M], mdt)
        wv_sb = const_pool.tile([128, 8, M], mdt)
        for w_sb, w_dr in ((wk_sb, wkT), (wq_sb, wqT)):
            nc.sync.dma_start(
                out=w_sb[:, :, :], in_=w_dr.rearrange("(kc p) m -> p kc m", p=128)
            )
        wo_sb = const_pool.tile([128, 2, D], mdt)
        bq_sb = const_pool.tile([128, 2], f32)
        bk_sb = const_pool.tile([128, 2], f32)
        nc.sync.dma_start(out=bq_sb[:, :], in_=bq.rearrange("(mt p) -> p mt", p=128))
        nc.sync.dma_start(out=bk_sb[:, :], in_=bk.rearrange("(mt p) -> p mt", p=128))
        bv_sb = const_pool.tile([1, M], mdt)
        nc.sync.dma_start(out=bv_sb[:, :], in_=bv.rearrange("(a m) -> a m", a=1))
        ones_sb = const_pool.tile([1, 128], mdt)
        nc.vector.memset(ones_sb[:, :], 1.0)
        ones32_sb = const_pool.tile([1, 64], f32r)
        nc.sync.dma_start(out=ones32_sb[:, :], in_=ones32_dr[:, :])

        qpT = const_pool.tile([128, 2, S], mdt)   # [p, mt, s]
        kpT = const_pool.tile([128, 2, S], mdt)
        vp = const_pool.tile([128, KT, HC * 65], mdt)
        attnT = const_pool.tile([128, 2, S], mdt)

        nc.vector.memset(
            vp[:, :, :].rearrange("p kt (h c) -> p kt h c", c=65)[:, :, :, 64:65], 1.0
        )

        # warm the ACT exp table during the prologue DMAs
        warm_sb = const_pool.tile([128, 8], f32)
        nc.vector.memset(warm_sb[:, :], 0.0)
        nc.scalar.activation(out=warm_sb[:, :], in_=warm_sb[:, :], func=EXP)

        # ---- projection / attention helpers ----
        def load_half(x_dr, half, pool=None):
            fs = half * 1024
            chunks = []
            for kc in range(8):
                xt = (pool or x_pool).tile([128, 1024], mdt, tag="xchunk")
                nc.sync.dma_start(
                    out=xt[:, :], in_=x_dr[kc * 128:(kc + 1) * 128, fs:fs + 1024]
                )
                chunks.append(xt)
            return chunks

        def proj_nch(chunks, w_sb, b_sb, p_sb, mt, half, nch):
            # one 512-token column block of qpT/kpT: 8 accumulating matmuls
            fs = half * 1024
            ns = nch * 512
            ps = ps_b.tile([128, 512], f32, tag="ps_small", name="ps_p")
            for kc in range(8):
                nc.tensor.matmul(
                    ps[:, :],
                    w_sb[:, kc, mt * 128:(mt + 1) * 128],
                    chunks[kc][:, ns:ns + 512],
                    start=(kc == 0),
                    stop=(kc == 7),
                )
            nc.vector.tensor_scalar_add(
                out=p_sb[:, mt, fs + ns:fs + ns + 512],
                in0=ps[:, :],
                scalar1=b_sb[:, mt:mt + 1],
            )

        def vp_group(chunks, kt):
            st = kt % 8
            ps = ps_b.tile([128, 256], f32, tag="ps_small", name="ps_v")
            for kc in range(8):
                nc.tensor.matmul(
                    ps[:, 0:M],
                    chunks[kc][:, st * 128:(st + 1) * 128],
                    wv_sb[:, kc, :],
                    start=(kc == 0),
                    stop=False,
                )
            nc.tensor.matmul(
                ps[:, 0:M],
                ones_sb[0:1, 0:128],
                bv_sb[0:1, :],
                start=False,
                stop=True,
            )
            nc.vector.tensor_copy(
                out=vp[:, kt, :].rearrange("p (h c) -> p h c", c=65)[:, :, 0:64],
                in_=ps[:, 0:M].rearrange("p (h c) -> p h c", c=64),
            )

        def flush_head(pair, qq, u, hh):
            # NOTE: u must be fully copied out of PSUM before the rb matmul —
            # rb's ring slot may wrap onto u's, and a later read of u would
            # deadlock against rb's WAR dependency.
            qs = qq * 512
            rs = r_pool.tile([1, 512], f32r, tag="rs")
            with nc.allow_low_precision(reason="softmax denom"):
                nc.vector.tensor_copy(out=rs[:, :], in_=u[64:65, :])
            us = r_pool.tile([64, 512], f32, tag="us")
            nc.vector.tensor_copy(out=us[:, :], in_=u[0:64, :])
            rb = ps_b.tile([64, 512], f32, tag="ps_small", name="rb")
            nc.tensor.matmul(
                rb[0:64, :], ones32_sb[0:1, 0:64], rs[0:1, :], start=True, stop=True
            )
            rbs = r_pool.tile([64, 512], f32, tag="rbs")
            nc.vector.reciprocal_approx_fast(out=rbs[:, :], in_=rb[0:64, :])
            with nc.allow_low_precision(reason="softmax normalize"):
                nc.vector.tensor_tensor(
                    out=attnT[hh * 64:hh * 64 + 64, pair, qs:qs + 512],
                    in0=us[0:64, :],
                    in1=rbs[0:64, :],
                    op=mybir.AluOpType.mult,
                )

        def outproj_stile(sg):
            ot = o_pool.tile([128, D], f32, name="ot")
            for nch in range(2):
                ns = nch * 512
                po = ps_b.tile([128, 512], f32, tag="ps_small", name="po")
                for kc in range(2):
                    nc.tensor.matmul(
                        po[:, :],
                        attnT[:, kc, sg * 128:(sg + 1) * 128],
                        wo_sb[:, kc, ns:ns + 512],
                        start=(kc == 0),
                        stop=(kc == 1),
                    )
                nc.vector.tensor_copy(out=ot[:, ns:ns + 512], in_=po[:, :])
            nc.sync.dma_start(out=out[sg * 128:(sg + 1) * 128, :], in_=ot[:, :])

        # ---- prologue: minimal path to the first score matmul ----
        chunks_k = load_half(xkT, 0)
        proj_nch(chunks_k, wk_sb, bk_sb, kpT, 0, 0, 0)
        proj_nch(chunks_k, wk_sb, bk_sb, kpT, 0, 0, 1)
        chunks_q = load_half(xqT, 0)
        proj_nch(chunks_q, wq_sb, bq_sb, qpT, 0, 0, 0)   # queries 0-511
        nc.sync.dma_start(
            out=wv_sb[:, :, :], in_=wvT.rearrange("(kc p) m -> p kc m", p=128)
        )
        chunks_k2 = load_half(xkT, 1)
        chunks_v = load_half(xvT, 0, pool=xv_pool)
        chunks_v2 = load_half(xvT, 1, pool=xv_pool)
        chunks_q2 = load_half(xqT, 1)
        nc.sync.dma_start(
            out=wo_sb[:, :, :], in_=woT.rearrange("(kc p) j -> p kc j", p=128)
        )

        # ---- filler queues ----
        # mand: order-critical chain (V-proj groups -> PV -> flush -> outproj)
        # bg:   independent projection work with (ready, deadline) windows
        mand = deque()   # items: dict(ready, cost, fn, after)
        bg = deque()     # items: dict(ready, deadline, cost, fn)
        pe_ns = [0.0]
        u_tiles = {}
        ets = {}

        pv_unlock = {0: 0}   # seg -> iteration when its PV chain may start

        def m_item(ready, cost, fn, after=None, gate_seg=None, front=False):
            it = {"ready": ready, "cost": cost, "fn": fn, "after": after,
                  "gate_seg": gate_seg}
            if front:
                mand.appendleft(it)
            else:
                mand.append(it)

        def b_item(ready, deadline, cost, fn):
            bg.append({"ready": ready, "deadline": deadline, "cost": cost, "fn": fn})

        def emit_scores_exp(seg, kt):
            pair, qq = divmod(seg, QQ)
            qs, ks = qq * 512, kt * 128
            sc = ps_a.tile([128, 2, 512], f32, tag="ps_main", name="sc")
            for hh in range(2):
                po = hh * 64
                nc.tensor.matmul(
                    sc[:, hh, :],
                    kpT[po:po + 64, pair, ks:ks + 128],
                    qpT[po:po + 64, pair, qs:qs + 512],
                    start=True,
                    stop=True,
                )
            et = e_pool.tile([128, 2, 512], mdt, tag="et", name="et")
            nc.scalar.activation(out=et[:, :, :], in_=sc[:, :, :], func=EXP)
            ets[(seg, kt)] = et
            pe_ns[0] += COST_SCORES

        def emit_pv(seg, kt):
            pair = seg // QQ
            if kt == 0:
                u_tiles[seg] = [
                    ps_b.tile([65, 512], f32, tag="ps_small", name=f"u_{seg}_{h}")
                    for h in range(2)
                ]
            et = ets.pop((seg, kt))
            for hh in range(2):
                h = 2 * pair + hh
                nc.tensor.matmul(
                    u_tiles[seg][hh][0:65, :],
                    vp[:, kt, h * 65:(h + 1) * 65],
                    et[:, hh, :],
                    start=(kt == 0),
                    stop=(kt == KT - 1),
                )

        def push_flushes(seg, g):
            # flushes go to the queue FRONT: the next segment's PV chain is
            # gated on them (only 2 u accumulators may be live in the PSUM
            # ring at once, else the rb allocation deadlocks on a slot whose
            # holder transitively depends on rb).
            pair, qq = divmod(seg, QQ)
            for hh in (1, 0):
                def fl(p=pair, q=qq, h=hh, s=seg):
                    flush_head(p, q, u_tiles[s][h], h)

                def unlock(g2, s=seg, p=pair, q=qq, h=hh):
                    if h == 1:
                        pv_unlock[s + 1] = g2 + 1
                        if p == 1:
                            push_outproj(q, g2)

                m_item(g + 1, COST_RB, fl, after=unlock, front=True)

        def push_outproj(qq, g):
            for i in range(4):
                m_item(g + 1 + i, COST_OUT,
                       (lambda sg=qq * 4 + i: outproj_stile(sg)))

        # V-projection groups + the PVs of segment 0, in consumption order.
        for kt in range(KT):
            ready = (5 + kt) if kt < 8 else (12 + (kt - 8))
            m_item(ready, COST_VP,
                   (lambda k=kt: vp_group(chunks_v if k < 8 else chunks_v2, k)))
            after = (lambda g: push_flushes(0, g)) if kt == KT - 1 else None
            m_item(ready, COST_PV, (lambda k=kt: emit_pv(0, k)), after=after,
                   gate_seg=0 if kt == 0 else None)

        # background projection fillers (x chunks stay resident in the
        # 32-slot ring, so mt1 reuses them — no re-streaming)
        b_item(0, 10, COST_PROJ,
               lambda: proj_nch(chunks_q, wq_sb, bq_sb, qpT, 0, 0, 1))
        b_item(3, 8, COST_PROJ,
               lambda: proj_nch(chunks_k2, wk_sb, bk_sb, kpT, 0, 1, 0))
        b_item(4, 9, COST_PROJ,
               lambda: proj_nch(chunks_k2, wk_sb, bk_sb, kpT, 0, 1, 1))
        b_item(16, 26, COST_PROJ,
               lambda: proj_nch(chunks_q2, wq_sb, bq_sb, qpT, 0, 1, 0))
        b_item(18, 28, COST_PROJ,
               lambda: proj_nch(chunks_q2, wq_sb, bq_sb, qpT, 0, 1, 1))
        for j, (half, nch) in enumerate(((0, 0), (0, 1), (1, 0), (1, 1))):
            b_item(24 + 2 * j, 52 + 2 * j, COST_PROJ,
                   (lambda h=half, n=nch:
                    proj_nch(chunks_k if h == 0 else chunks_k2,
                             wk_sb, bk_sb, kpT, 1, h, n)))
        for j, (half, nch) in enumerate(((0, 0), (0, 1), (1, 0), (1, 1))):
            b_item(32 + 2 * j, 56 + 2 * j, COST_PROJ,
                   (lambda h=half, n=nch:
                    proj_nch(chunks_q if h == 0 else chunks_q2,
                             wq_sb, bq_sb, qpT, 1, h, n)))

        def m_ready(it, g):
            if it["ready"] > g:
                return False
            gs = it["gate_seg"]
            if gs is not None and (gs not in pv_unlock or pv_unlock[gs] > g):
                return False
            return True

        def pump(g, budget):
            progressed = True
            while progressed:
                progressed = False
                if mand and m_ready(mand[0], g) and (
                    pe_ns[0] + mand[0]["cost"] <= budget
                ):
                    it = mand.popleft()
                    it["fn"]()
                    pe_ns[0] += it["cost"]
                    if it["after"]:
                        it["after"](g)
                    progressed = True
                    continue
                if bg and bg[0]["ready"] <= g and (
                    pe_ns[0] + bg[0]["cost"] <= budget or g >= bg[0]["deadline"]
                ):
                    it = bg.popleft()
                    it["fn"]()
                    pe_ns[0] += it["cost"]
                    progressed = True

        # ---- the ACT-paced attention loop ----
        for g in range(NSEG * KT):
            seg, kt = divmod(g, KT)
            emit_scores_exp(seg, kt)
            if seg > 0:
                after = (lambda gg, s=seg: push_flushes(s, gg)) if kt == KT - 1 else None
                m_item(g + 1, COST_PV, (lambda s=seg, k=kt: emit_pv(s, k)),
                       after=after, gate_seg=seg if kt == 0 else None)
            pump(g, (g + 1) * PACE + SLOP)

        # ---- tail: drain remaining work ----
        g = NSEG * KT
        while mand or bg:
            pump(g, float("inf"))
            g += 1

    nc.compile()
    return nc


def _get_compiled():
    global _compiled
    if _compiled is None:
        _compiled = _build_program()
    return _compiled


def _make_in_maps(q, k, v, in_proj_w, in_proj_b, out_proj_w):
    import ml_dtypes

    mdt_np = np.dtype(ml_dtypes.bfloat16) if MM_DT == "bfloat16" else np.float32

    def cvt(a):
        return np.ascontiguousarray(a).astype(mdt_np)

    xT = {}
    for b in range(B):
        xT[b] = (
            cvt(q[:, b, :].T),
            cvt(k[:, b, :].T),
            cvt(v[:, b, :].T),
        )
    scale = 1.0 / math.sqrt(DK)
    in_maps = []
    for c in range(N_CORES):
        b, g = divmod(c, HC)
        cols = slice(g * M, (g + 1) * M)
        in_maps.append({
            "xqT": xT[b][0],
            "xkT": xT[b][1],
            "xvT": xT[b][2],
            "wqT": cvt((in_proj_w[0 * D:1 * D][cols] * scale).T),
            "wkT": cvt(in_proj_w[1 * D:2 * D][cols].T),
            "wvT": cvt(in_proj_w[2 * D:3 * D][cols].T),
            "bq": np.ascontiguousarray(in_proj_b[0 * D:1 * D][cols] * scale),
            "bk": np.ascontiguousarray(in_proj_b[1 * D:2 * D][cols]),
            "bv": cvt(in_proj_b[2 * D:3 * D][cols]),
            "woT": cvt(out_proj_w[:, g * M:(g + 1) * M].T),
            "ones32": np.ones((1, 64), dtype=np.float32),
        })
    return in_maps


def kernel(q, k, v, in_proj_w, in_proj_b, out_proj_w, out_proj_b):
    from concourse.bass_utils import run_bass_kernel_spmd

    q = np.asarray(q, dtype=np.float32)
    k = np.asarray(k, dtype=np.float32)
    v = np.asarray(v, dtype=np.float32)
    in_proj_w = np.asarray(in_proj_w, dtype=np.float32)
    in_proj_b = np.asarray(in_proj_b, dtype=np.float32)
    out_proj_w = np.asarray(out_proj_w, dtype=np.float32)
    out_proj_b = np.asarray(out_proj_b, dtype=np.float32)

    nc = _get_compiled()
    in_maps = _make_in_maps(q, k, v, in_proj_w, in_proj_b, out_proj_w)

    res = run_bass_kernel_spmd(nc, in_maps, core_ids=list(range(N_CORES)))

    out = np.broadcast_to(out_proj_b.astype(np.float32), (S, B, D)).copy()
    for c in range(N_CORES):
        out[:, c // HC, :] += res.results[c]["out"]
    return out


# revision 46
# speedup vs baseline: 1.0509x; 1.0025x over previous
"""Multi-head self-attention (S=2048, B=2, D=1024, H=16) on 8 TRN2 NeuronCores.

Sharding: core c handles batch b = c//4 and head-quad g = c%4 (4 heads of 64).
Megatron-style: in_proj column-sliced, out_proj row-sliced; host sums the 8
partial outputs and adds out_proj bias.

Per-core dataflow (matmul inputs bf16, accumulation fp32):
  - host supplies x^T (D-major) activations and pre-transposed weight slices
  - qpT/kpT computed head-major (m on partitions, seq on free)
  - vp computed seq-major with an interleaved ones column per head (65-wide
    blocks) so the PV matmul also produces softmax row-sums on partition 64
  - scores per (head-pair, 512-query-chunk, key-tile): the two heads' K=64
    matmuls land on disjoint PE row-groups (base partitions 0/64) and run
    concurrently; exp on ACT reads the pair in one op
  - the attention loop is paced by the ACT exp (~1.1us per key-tile); all
    other PE work (V projection, mt1 projections, out-projection, flushes)
    is drip-fed as "filler" between score matmuls via an explicit
    budget-tracked queue so neither engine sees a long stall
  - normalization: K=1 matmul broadcasts the row-sums, DVE divides reading
    the PV accumulator straight from PSUM
  - out-projection on device from attn^T; bias + cross-core reduction on host
"""

import math
from collections import deque
from contextlib import ExitStack

import numpy as np

S = 2048
B = 2
D = 1024
H = 16
DK = 64
HC = 4          # heads per core
M = HC * DK     # 256 head-dim columns per core
N_CORES = 8
KT = S // 128   # 16 key tiles
QQ = 4          # 512-wide query chunks
NSEG = 2 * QQ   # (pair, qq) segments

MM_DT = "bfloat16"

# pacing ledger: target PE-ns budget per attention iteration
PACE = 1420.0
SLOP = 700.0
COST_SCORES = 360.0
COST_PV = 480.0
COST_VP = 990.0
COST_PROJ = 1740.0
COST_RB = 450.0
COST_OUT = 900.0

_compiled = None


def _build_program():
    import concourse.tile as tile
    from concourse import mybir, bacc

    f32 = mybir.dt.float32
    f32r = mybir.dt.float32r
    mdt = getattr(mybir.dt, MM_DT)
    EXP = mybir.ActivationFunctionType.Exp

    nc = bacc.Bacc("TRN2", target_bir_lowering=False, debug=False)

    xqT = nc.dram_tensor("xqT", [D, S], mdt, kind="ExternalInput").ap()
    xkT = nc.dram_tensor("xkT", [D, S], mdt, kind="ExternalInput").ap()
    xvT = nc.dram_tensor("xvT", [D, S], mdt, kind="ExternalInput").ap()
    wqT = nc.dram_tensor("wqT", [D, M], mdt, kind="ExternalInput").ap()
    wkT = nc.dram_tensor("wkT", [D, M], mdt, kind="ExternalInput").ap()
    wvT = nc.dram_tensor("wvT", [D, M], mdt, kind="ExternalInput").ap()
    bq = nc.dram_tensor("bq", [M], f32, kind="ExternalInput").ap()
    bk = nc.dram_tensor("bk", [M], f32, kind="ExternalInput").ap()
    bv = nc.dram_tensor("bv", [M], mdt, kind="ExternalInput").ap()
    woT = nc.dram_tensor("woT", [M, D], mdt, kind="ExternalInput").ap()
    ones32_dr = nc.dram_tensor("ones32", [1, 64], f32r, kind="ExternalInput").ap()
    out = nc.dram_tensor("out", [S, D], f32, kind="ExternalOutput").ap()

    with tile.TileContext(nc) as tc, ExitStack() as ctx:
        const_pool = ctx.enter_context(tc.tile_pool(name="const", bufs=1))
        x_pool = ctx.enter_context(tc.tile_pool(name="x", bufs=32))
        xv_pool = ctx.enter_context(tc.tile_pool(name="xv", bufs=16))
        e_pool = ctx.enter_context(tc.tile_pool(name="e", bufs=16))
        o_pool = ctx.enter_context(tc.tile_pool(name="o", bufs=2))
        r_pool = ctx.enter_context(tc.tile_pool(name="r", bufs=2))
        ps_a = ctx.enter_context(tc.tile_pool(name="ps_a", bufs=2, space="PSUM"))
        ps_b = ctx.enter_context(tc.tile_pool(name="ps_b", bufs=4, space="PSUM"))

        # ---- persistent SBUF tensors ----
        # wo is loaded LAST (after all x tensors) — it isn't needed until the
        # first out-projection, ~100us in.
        wq_sb = const_pool.tile([128, 8, M], mdt)
        wk_sb = const_pool.tile([128, 8, M], mdt)
        wv_sb = const_pool.tile([128, 8, M], mdt)
        for w_sb, w_dr in ((wk_sb, wkT), (wq_sb, wqT), (wv_sb, wvT)):
            nc.sync.dma_start(
                out=w_sb[:, :, :], in_=w_dr.rearrange("(kc p) m -> p kc m", p=128)
            )
        wo_sb = const_pool.tile([128, 2, D], mdt)
        bq_sb = const_pool.tile([128, 2], f32)
        bk_sb = const_pool.tile([128, 2], f32)
        nc.sync.dma_start(out=bq_sb[:, :], in_=bq.rearrange("(mt p) -> p mt", p=128))
        nc.sync.dma_start(out=bk_sb[:, :], in_=bk.rearrange("(mt p) -> p mt", p=128))
        bv_sb = const_pool.tile([1, M], mdt)
        nc.sync.dma_start(out=bv_sb[:, :], in_=bv.rearrange("(a m) -> a m", a=1))
        ones_sb = const_pool.tile([1, 128], mdt)
        nc.vector.memset(ones_sb[:, :], 1.0)
        ones32_sb = const_pool.tile([1, 64], f32r)
        nc.sync.dma_start(out=ones32_sb[:, :], in_=ones32_dr[:, :])

        qpT = const_pool.tile([128, 2, S], mdt)   # [p, mt, s]
        kpT = const_pool.tile([128, 2, S], mdt)
        vp = const_pool.tile([128, KT, HC * 65], mdt)
        attnT = const_pool.tile([128, 2, S], mdt)

        nc.vector.memset(
            vp[:, :, :].rearrange("p kt (h c) -> p kt h c", c=65)[:, :, :, 64:65], 1.0
        )

        # warm the ACT exp table during the prologue DMAs
        warm_sb = const_pool.tile([128, 8], f32)
        nc.vector.memset(warm_sb[:, :], 0.0)
        nc.scalar.activation(out=warm_sb[:, :], in_=warm_sb[:, :], func=EXP)

        # ---- projection / attention helpers ----
        def load_half(x_dr, half, pool=None):
            fs = half * 1024
            chunks = []
            for kc in range(8):
                xt = (pool or x_pool).tile([128, 1024], mdt, tag="xchunk")
                nc.sync.dma_start(
                    out=xt[:, :], in_=x_dr[kc * 128:(kc + 1) * 128, fs:fs + 1024]
                )
                chunks.append(xt)
            return chunks

        def proj_nch(chunks, w_sb, b_sb, p_sb, mt, half, nch):
            # one 512-token column block of qpT/kpT: 8 accumulating matmuls
            fs = half * 1024
            ns = nch * 512
            ps = ps_b.tile([128, 512], f32, tag="ps_small", name="ps_p")
            for kc in range(8):
                nc.tensor.matmul(
                    ps[:, :],
                    w_sb[:, kc, mt * 128:(mt + 1) * 128],
                    chunks[kc][:, ns:ns + 512],
                    start=(kc == 0),
                    stop=(kc == 7),
                )
            nc.vector.tensor_scalar_add(
                out=p_sb[:, mt, fs + ns:fs + ns + 512],
                in0=ps[:, :],
                scalar1=b_sb[:, mt:mt + 1],
            )

        def vp_group(chunks, kt):
            st = kt % 8
            ps = ps_b.tile([128, 256], f32, tag="ps_small", name="ps_v")
            for kc in range(8):
                nc.tensor.matmul(
                    ps[:, 0:M],
                    chunks[kc][:, st * 128:(st + 1) * 128],
                    wv_sb[:, kc, :],
                    start=(kc == 0),
                    stop=False,
                )
            nc.tensor.matmul(
                ps[:, 0:M],
                ones_sb[0:1, 0:128],
                bv_sb[0:1, :],
                start=False,
                stop=True,
            )
            nc.vector.tensor_copy(
                out=vp[:, kt, :].rearrange("p (h c) -> p h c", c=65)[:, :, 0:64],
                in_=ps[:, 0:M].rearrange("p (h c) -> p h c", c=64),
            )

        def flush_head(pair, qq, u, hh):
            # NOTE: u must be fully copied out of PSUM before the rb matmul —
            # rb's ring slot may wrap onto u's, and a later read of u would
            # deadlock against rb's WAR dependency.
            qs = qq * 512
            rs = r_pool.tile([1, 512], f32r, tag="rs")
            with nc.allow_low_precision(reason="softmax denom"):
                nc.vector.tensor_copy(out=rs[:, :], in_=u[64:65, :])
            us = r_pool.tile([64, 512], f32, tag="us")
            nc.vector.tensor_copy(out=us[:, :], in_=u[0:64, :])
            rb = ps_b.tile([64, 512], f32, tag="ps_small", name="rb")
            nc.tensor.matmul(
                rb[0:64, :], ones32_sb[0:1, 0:64], rs[0:1, :], start=True, stop=True
            )
            rbs = r_pool.tile([64, 512], f32, tag="rbs")
            nc.vector.reciprocal_approx_fast(out=rbs[:, :], in_=rb[0:64, :])
            with nc.allow_low_precision(reason="softmax normalize"):
                nc.vector.tensor_tensor(
                    out=attnT[hh * 64:hh * 64 + 64, pair, qs:qs + 512],
                    in0=us[0:64, :],
                    in1=rbs[0:64, :],
                    op=mybir.AluOpType.mult,
                )

        def outproj_stile(sg):
            ot = o_pool.tile([128, D], f32, name="ot")
            for nch in range(2):
                ns = nch * 512
                po = ps_b.tile([128, 512], f32, tag="ps_small", name="po")
                for kc in range(2):
                    nc.tensor.matmul(
                        po[:, :],
                        attnT[:, kc, sg * 128:(sg + 1) * 128],
                        wo_sb[:, kc, ns:ns + 512],
                        start=(kc == 0),
                        stop=(kc == 1),
                    )
                nc.vector.tensor_copy(out=ot[:, ns:ns + 512], in_=po[:, :])
            nc.sync.dma_start(out=out[sg * 128:(sg + 1) * 128, :], in_=ot[:, :])

        # ---- prologue: minimal path to the first score matmul ----
        chunks_k = load_half(xkT, 0)
        proj_nch(chunks_k, wk_sb, bk_sb, kpT, 0, 0, 0)
        proj_nch(chunks_k, wk_sb, bk_sb, kpT, 0, 0, 1)
        chunks_q = load_half(xqT, 0)
        proj_nch(chunks_q, wq_sb, bq_sb, qpT, 0, 0, 0)   # queries 0-511
        chunks_k2 = load_half(xkT, 1)
        chunks_v = load_half(xvT, 0, pool=xv_pool)
        chunks_v2 = load_half(xvT, 1, pool=xv_pool)
        chunks_q2 = load_half(xqT, 1)
        nc.sync.dma_start(
            out=wo_sb[:, :, :], in_=woT.rearrange("(kc p) j -> p kc j", p=128)
        )

        # ---- filler queues ----
        # mand: order-critical chain (V-proj groups -> PV -> flush -> outproj)
        # bg:   independent projection work with (ready, deadline) windows
        mand = deque()   # items: dict(ready, cost, fn, after)
        bg = deque()     # items: dict(ready, deadline, cost, fn)
        pe_ns = [0.0]
        u_tiles = {}
        ets = {}

        pv_unlock = {0: 0}   # seg -> iteration when its PV chain may start

        def m_item(ready, cost, fn, after=None, gate_seg=None, front=False):
            it = {"ready": ready, "cost": cost, "fn": fn, "after": after,
                  "gate_seg": gate_seg}
            if front:
                mand.appendleft(it)
            else:
                mand.append(it)

        def b_item(ready, deadline, cost, fn):
            bg.append({"ready": ready, "deadline": deadline, "cost": cost, "fn": fn})

        def emit_scores_exp(seg, kt):
            pair, qq = divmod(seg, QQ)
            qs, ks = qq * 512, kt * 128
            sc = ps_a.tile([128, 2, 512], f32, tag="ps_main", name="sc")
            for hh in range(2):
                po = hh * 64
                nc.tensor.matmul(
                    sc[:, hh, :],
                    kpT[po:po + 64, pair, ks:ks + 128],
                    qpT[po:po + 64, pair, qs:qs + 512],
                    start=True,
                    stop=True,
                )
            et = e_pool.tile([128, 2, 512], mdt, tag="et", name="et")
            nc.scalar.activation(out=et[:, :, :], in_=sc[:, :, :], func=EXP)
            ets[(seg, kt)] = et
            pe_ns[0] += COST_SCORES

        def emit_pv(seg, kt):
            pair = seg // QQ
            if kt == 0:
                u_tiles[seg] = [
                    ps_b.tile([65, 512], f32, tag="ps_small", name=f"u_{seg}_{h}")
                    for h in range(2)
                ]
            et = ets.pop((seg, kt))
            for hh in range(2):
                h = 2 * pair + hh
                nc.tensor.matmul(
                    u_tiles[seg][hh][0:65, :],
                    vp[:, kt, h * 65:(h + 1) * 65],
                    et[:, hh, :],
                    start=(kt == 0),
                    stop=(kt == KT - 1),
                )

        def push_flushes(seg, g):
            # flushes go to the queue FRONT: the next segment's PV chain is
            # gated on them (only 2 u accumulators may be live in the PSUM
            # ring at once, else the rb allocation deadlocks on a slot whose
            # holder transitively depends on rb).
            pair, qq = divmod(seg, QQ)
            for hh in (1, 0):
                def fl(p=pair, q=qq, h=hh, s=seg):
                    flush_head(p, q, u_tiles[s][h], h)

                def unlock(g2, s=seg, p=pair, q=qq, h=hh):
                    if h == 1:
                        pv_unlock[s + 1] = g2 + 1
                        if p == 1:
                            push_outproj(q, g2)

                m_item(g + 1, COST_RB, fl, after=unlock, front=True)

        def push_outproj(qq, g):
            # no stagger: mid-stream the budget paces these anyway, and the
            # final quarter's stiles must drain immediately after the last
            # flush or the PE goes HAM-cold in the tail
            for i in range(4):
                m_item(g + 1, COST_OUT,
                       (lambda sg=qq * 4 + i: outproj_stile(sg)))

        # V-projection groups + the PVs of segment 0, in consumption order.
        for kt in range(KT):
            ready = (5 + kt) if kt < 8 else (12 + (kt - 8))
            m_item(ready, COST_VP,
                   (lambda k=kt: vp_group(chunks_v if k < 8 else chunks_v2, k)))
            after = (lambda g: push_flushes(0, g)) if kt == KT - 1 else None
            m_item(ready, COST_PV, (lambda k=kt: emit_pv(0, k)), after=after,
                   gate_seg=0 if kt == 0 else None)

        # background projection fillers (x chunks stay resident in the
        # 32-slot ring, so mt1 reuses them — no re-streaming)
        b_item(0, 10, COST_PROJ,
               lambda: proj_nch(chunks_q, wq_sb, bq_sb, qpT, 0, 0, 1))
        b_item(3, 8, COST_PROJ,
               lambda: proj_nch(chunks_k2, wk_sb, bk_sb, kpT, 0, 1, 0))
        b_item(4, 9, COST_PROJ,
               lambda: proj_nch(chunks_k2, wk_sb, bk_sb, kpT, 0, 1, 1))
        b_item(16, 26, COST_PROJ,
               lambda: proj_nch(chunks_q2, wq_sb, bq_sb, qpT, 0, 1, 0))
        b_item(18, 28, COST_PROJ,
               lambda: proj_nch(chunks_q2, wq_sb, bq_sb, qpT, 0, 1, 1))
        for j, (half, nch) in enumerate(((0, 0), (0, 1), (1, 0), (1, 1))):
            b_item(24 + 2 * j, 52 + 2 * j, COST_PROJ,
                   (lambda h=half, n=nch:
                    proj_nch(chunks_k if h == 0 else chunks_k2,
                             wk_sb, bk_sb, kpT, 1, h, n)))
        for j, (half, nch) in enumerate(((0, 0), (0, 1), (1, 0), (1, 1))):
            b_item(32 + 2 * j, 56 + 2 * j, COST_PROJ,
                   (lambda h=half, n=nch:
                    proj_nch(chunks_q if h == 0 else chunks_q2,
                             wq_sb, bq_sb, qpT, 1, h, n)))

        def m_ready(it, g):
            if it["ready"] > g:
                return False
            gs = it["gate_seg"]
            if gs is not None and (gs not in pv_unlock or pv_unlock[gs] > g):
                return False
            return True

        def pump(g, budget):
            progressed = True
            while progressed:
                progressed = False
                if mand and m_ready(mand[0], g) and (
                    pe_ns[0] + mand[0]["cost"] <= budget
                ):
                    it = mand.popleft()
                    it["fn"]()
                    pe_ns[0] += it["cost"]
                    if it["after"]:
                        it["after"](g)
                    progressed = True
                    continue
                if bg and bg[0]["ready"] <= g and (
                    pe_ns[0] + bg[0]["cost"] <= budget or g >= bg[0]["deadline"]
                ):
                    it = bg.popleft()
                    it["fn"]()
                    pe_ns[0] += it["cost"]
                    progressed = True

        # ---- the ACT-paced attention loop ----
        for g in range(NSEG * KT):
            seg, kt = divmod(g, KT)
            emit_scores_exp(seg, kt)
            if seg > 0:
                after = (lambda gg, s=seg: push_flushes(s, gg)) if kt == KT - 1 else None
                m_item(g + 1, COST_PV, (lambda s=seg, k=kt: emit_pv(s, k)),
                       after=after, gate_seg=seg if kt == 0 else None)
            pump(g, (g + 1) * PACE + SLOP)

        # ---- tail: drain remaining work ----
        g = NSEG * KT
        while mand or bg:
            pump(g, float("inf"))
            g += 1

    nc.compile()
    return nc


def _get_compiled():
    global _compiled
    if _compiled is None:
        _compiled = _build_program()
    return _compiled


def _make_in_maps(q, k, v, in_proj_w, in_proj_b, out_proj_w):
    import ml_dtypes

    mdt_np = np.dtype(ml_dtypes.bfloat16) if MM_DT == "bfloat16" else np.float32

    def cvt(a):
        return np.ascontiguousarray(a).astype(mdt_np)

    xT = {}
    for b in range(B):
        xT[b] = (
            cvt(q[:, b, :].T),
            cvt(k[:, b, :].T),
            cvt(v[:, b, :].T),
        )
    scale = 1.0 / math.sqrt(DK)
    in_maps = []
    for c in range(N_CORES):
        b, g = divmod(c, HC)
        cols = slice(g * M, (g + 1) * M)
        in_maps.append({
            "xqT": xT[b][0],
            "xkT": xT[b][1],
            "xvT": xT[b][2],
            "wqT": cvt((in_proj_w[0 * D:1 * D][cols] * scale).T),
            "wkT": cvt(in_proj_w[1 * D:2 * D][cols].T),
            "wvT": cvt(in_proj_w[2 * D:3 * D][cols].T),
            "bq": np.ascontiguousarray(in_proj_b[0 * D:1 * D][cols] * scale),
            "bk": np.ascontiguousarray(in_proj_b[1 * D:2 * D][cols]),
            "bv": cvt(in_proj_b[2 * D:3 * D][cols]),
            "woT": cvt(out_proj_w[:, g * M:(g + 1) * M].T),
            "ones32": np.ones((1, 64), dtype=np.float32),
        })
    return in_maps


def kernel(q, k, v, in_proj_w, in_proj_b, out_proj_w, out_proj_b):
    from concourse.bass_utils import run_bass_kernel_spmd

    q = np.asarray(q, dtype=np.float32)
    k = np.asarray(k, dtype=np.float32)
    v = np.asarray(v, dtype=np.float32)
    in_proj_w = np.asarray(in_proj_w, dtype=np.float32)
    in_proj_b = np.asarray(in_proj_b, dtype=np.float32)
    out_proj_w = np.asarray(out_proj_w, dtype=np.float32)
    out_proj_b = np.asarray(out_proj_b, dtype=np.float32)

    nc = _get_compiled()
    in_maps = _make_in_maps(q, k, v, in_proj_w, in_proj_b, out_proj_w)

    res = run_bass_kernel_spmd(nc, in_maps, core_ids=list(range(N_CORES)))

    out = np.broadcast_to(out_proj_b.astype(np.float32), (S, B, D)).copy()
    for c in range(N_CORES):
        out[:, c // HC, :] += res.results[c]["out"]
    return out
